# revision 31
# baseline (speedup 1.0000x reference)
"""Trainium2 Bass kernel for the 3D attention layer.

Math (reference):
    x[b,s,t] = inputs[b,t,s] * mask[b,t,s]
    z[b,s,u] = sum_t x[b,s,t] * W[s,t,u] + bias[s,u]
    e = exp(tanh(z))
    att[b,s,u] = e / (sum_u' e[b,s,u'] + eps)
    out[b,u] = sum_s att[b,s,u] * x[b,s,u]

Distribution: S (=2048) is sharded across 8 NeuronCores (256 positions per
core).  Each core computes a partial out[b,u] over its S-shard; the host sums
the 8 partials (the only cross-core reduction, 64 KB per core).

Per-core layout ("t-layout"): tiles are [partition=(js,t) or (js,u)][free=b]
where 4 consecutive s-positions (js=0..3) are packed into the 128 partitions
(4*32).  The per-position (T,T) matmuls become block-diagonal 128x128
matmuls with b=512 as the moving free dim.  The softmax denominator
(a partition-direction reduction) and its broadcast back across partitions
are done on the TensorEngine with constant 0/1 selector matrices; the
reciprocal itself runs on the small stacked [4*gs,512] tensor only.

All compute-facing tensors are bf16 (TensorEngine full rate, DVE 2x mode);
PSUM accumulation is fp32.  Work is emitted in variable-size groups of
position-blocks (small groups first/last to fill/drain the pipeline).
"""

import sys

for _p in ("/opt/trn_rl_repo", "/opt/pypackages"):
    if _p not in sys.path:
        sys.path.insert(0, _p)

import numpy as np
import ml_dtypes

BF16 = ml_dtypes.bfloat16

# Problem shape (hardcoded per spec).
B, T, S = 512, 32, 2048
NCORES = 8
S_LOC = S // NCORES       # 256 positions per core
NBLK = S_LOC // 4         # 64 blocks of 4 positions
GSZ = 8                   # max blocks per pipeline group (selector capacity)
FD = B                    # moving free dim (batch)

# Group schedule: sums to NBLK; small groups at both ends for pipeline
# fill/drain, GSZ-sized groups in steady state.
SCHED = [2, 2, 4, 8, 8, 8, 8, 8, 8, 4, 2, 2]
assert sum(SCHED) == NBLK

_cache = {}


def _build_nc(
    sched=tuple(SCHED),
    bufs_x=3,
    bufs_e=3,
    bufs_z=2,
    bufs_att=3,
    exp_chunk=4096,
    mask_engine="gpsimd",
    rb_copy_engine="vector",
    mask_dve_frac=0.0,
    group_mask=False,
    attx_pool_tail=0,
    w_chunks=4,
    repeat=1,
    mask_pair=False,
    att_pair=False,
    rb_bufs=2,
    mask_dve_groups=5,
    fin_batch=False,
):
    """Build the single-core Bass graph (SPMD: same graph on all 8 cores)."""
    from contextlib import ExitStack

    import concourse.mybir as mybir
    from concourse import bacc
    from concourse.tile import TileContext

    sched = list(sched)
    assert sum(sched) == NBLK and max(sched) <= GSZ

    BF = mybir.dt.bfloat16
    F32 = mybir.dt.float32
    AF = mybir.ActivationFunctionType

    nc = bacc.Bacc(None, target_bir_lowering=False)
    x_d = nc.declare_dram_parameter("x", [NBLK, 128, FD], BF, isOutput=False)
    m_d = nc.declare_dram_parameter("m", [NBLK, 128, FD], BF, isOutput=False)
    w_d = nc.declare_dram_parameter("w", [128, NBLK * 128], BF, isOutput=False)
    bb_d = nc.declare_dram_parameter("bb", [128, NBLK], F32, isOutput=False)
    sd_d = nc.declare_dram_parameter("sd", [128, GSZ * 4 * GSZ], BF, isOutput=False)
    sc_d = nc.declare_dram_parameter("sc", [4 * GSZ, GSZ * 128], BF, isOutput=False)
    sf_d = nc.declare_dram_parameter("sf", [128, 32], BF, isOutput=False)
    o_d = nc.declare_dram_parameter("out", [32, FD], F32, isOutput=True)

    with TileContext(nc) as tc, ExitStack() as ctx:
        singles = ctx.enter_context(tc.tile_pool(name="singles", bufs=1))
        xp = ctx.enter_context(tc.tile_pool(name="xp", bufs=bufs_x))
        mp = ctx.enter_context(tc.tile_pool(name="mp", bufs=bufs_x))
        xmp = ctx.enter_context(tc.tile_pool(name="xmp", bufs=bufs_x))
        t1p = ctx.enter_context(tc.tile_pool(name="t1p", bufs=1))
        ep = ctx.enter_context(tc.tile_pool(name="ep", bufs=bufs_e))
        rp = ctx.enter_context(tc.tile_pool(name="rp", bufs=2))
        attp = ctx.enter_context(tc.tile_pool(name="attp", bufs=bufs_att))
        gtp = ctx.enter_context(tc.tile_pool(name="gtp", bufs=bufs_att))
        zpool = ctx.enter_context(tc.tile_pool(name="zp", bufs=bufs_z, space="PSUM"))
        dpool = ctx.enter_context(tc.tile_pool(name="dp", bufs=2, space="PSUM"))
        rbpool = ctx.enter_context(
            tc.tile_pool(name="rbp", bufs=rb_bufs, space="PSUM")
        )
        apool = ctx.enter_context(tc.tile_pool(name="ap", bufs=1, space="PSUM"))

        acc = apool.tile([32, FD], F32)
        mask_mul = (
            nc.gpsimd.tensor_mul if mask_engine == "gpsimd" else nc.vector.tensor_mul
        )

        def dma_in(k0, gs):
            xg = xp.tile([128, gs, FD], BF, tag="xg")
            nc.sync.dma_start(
                out=xg, in_=x_d[k0 : k0 + gs].rearrange("k p b -> p k b")
            )
            mg = mp.tile([128, gs, FD], BF, tag="mg")
            nc.sync.dma_start(
                out=mg, in_=m_d[k0 : k0 + gs].rearrange("k p b -> p k b")
            )
            return xg, mg

        # First-group input DMAs are issued before the (large) constant DMAs
        # so compute can start as soon as possible.
        first_dmas = dma_in(0, sched[0])

        # Constants, resident for the whole kernel.  W is split into chunks
        # so the first z-matmuls don't wait on the full 2 MB transfer.
        w_sb = singles.tile([128, NBLK * 128], BF)
        wc = NBLK // w_chunks
        for ci in range(w_chunks):
            nc.sync.dma_start(
                out=w_sb[:, ci * wc * 128 : (ci + 1) * wc * 128],
                in_=w_d[:, ci * wc * 128 : (ci + 1) * wc * 128],
            )
        bb_sb = singles.tile([128, NBLK], F32)
        nc.sync.dma_start(out=bb_sb, in_=bb_d[:, :])
        sd_sb = singles.tile([128, GSZ * 4 * GSZ], BF)
        nc.sync.dma_start(out=sd_sb, in_=sd_d[:, :])
        sc_sb = singles.tile([4 * GSZ, GSZ * 128], BF)
        nc.sync.dma_start(out=sc_sb, in_=sc_d[:, :])
        sf_sb = singles.tile([128, 32], BF)
        nc.sync.dma_start(out=sf_sb, in_=sf_d[:, :])

        def phase1(k0, gs, xg, mg, force_dve_mask=False):
            g_mask_mul = nc.vector.tensor_mul if force_dve_mask else mask_mul
            xm = xmp.tile([128, gs, FD], BF, tag="xm")
            t1 = t1p.tile([128, GSZ * FD], BF)
            eg = ep.tile([128, gs, FD], BF, tag="eg")
            den = dpool.tile([4 * gs, FD], F32, tag="den")

            if group_mask:
                # One TT over the whole group amortizes per-op overhead.
                mask_mul(
                    xm.rearrange("p k b -> p (k b)"),
                    xg.rearrange("p k b -> p (k b)"),
                    mg.rearrange("p k b -> p (k b)"),
                )
            elif mask_pair:
                xmf = xm.rearrange("p k b -> p (k b)")
                xgf = xg.rearrange("p k b -> p (k b)")
                mgf = mg.rearrange("p k b -> p (k b)")
                for j in range(0, gs, 2):
                    w2 = min(2, gs - j)
                    sl2 = slice(j * FD, (j + w2) * FD)
                    mask_mul(xmf[:, sl2], xgf[:, sl2], mgf[:, sl2])
            n_dve_mask = int(round(gs * mask_dve_frac))
            for j in range(gs):
                kk = k0 + j
                if not group_mask and not mask_pair:
                    mm = (
                        nc.vector.tensor_mul
                        if j < n_dve_mask
                        else g_mask_mul
                    )
                    mm(xm[:, j], xg[:, j], mg[:, j])
                z = zpool.tile([128, FD], F32)
                nc.tensor.matmul(
                    z,
                    w_sb[:, kk * 128 : (kk + 1) * 128],
                    xm[:, j],
                    start=True,
                    stop=True,
                )
                # t1 = tanh(z + bias)  (bias folded into the ACT affine stage)
                nc.scalar.activation(
                    t1[:, j * FD : (j + 1) * FD],
                    z,
                    AF.Tanh,
                    bias=bb_sb[:, kk : kk + 1],
                )

            done = 0
            while done < gs * FD:
                csz = min(exp_chunk, gs * FD - done)
                nc.scalar.activation(
                    eg.rearrange("p k b -> p (k b)")[:, done : done + csz],
                    t1[:, done : done + csz],
                    AF.Exp,
                )
                done += csz

            for j in range(gs):
                nc.tensor.matmul(
                    den,
                    sd_sb[:, j * 4 * GSZ : j * 4 * GSZ + 4 * gs],
                    eg[:, j],
                    start=(j == 0),
                    stop=(j == gs - 1),
                )
            return xm, eg, den

        def phase2(k0, gs, xm, eg, den, attx_pool=False):
            rf = rp.tile([4 * GSZ, FD], F32, tag="rf")
            nc.vector.reciprocal_approx_fast(rf[: 4 * gs], den)
            rb = rp.tile([4 * GSZ, FD], BF, tag="rb")
            if rb_copy_engine == "scalar":
                nc.scalar.copy(rb[: 4 * gs], rf[: 4 * gs])
            else:
                nc.vector.tensor_copy(rb[: 4 * gs], rf[: 4 * gs])

            attx_mul = nc.gpsimd.tensor_mul if attx_pool else nc.vector.tensor_mul
            if att_pair:
                egf = eg.rearrange("p k b -> p (k b)")
                xmf = xm.rearrange("p k b -> p (k b)")
                for j in range(0, gs, 2):
                    w2 = min(2, gs - j)
                    rbb = rbpool.tile([128, 2 * FD], F32, tag="rbb")
                    for i in range(w2):
                        nc.tensor.matmul(
                            rbb[:, i * FD : (i + 1) * FD],
                            sc_sb[: 4 * gs, (j + i) * 128 : (j + i + 1) * 128],
                            rb[: 4 * gs],
                            start=True,
                            stop=True,
                        )
                    sl2 = slice(j * FD, (j + w2) * FD)
                    att = attp.tile([128, 2 * FD], BF, tag="att")
                    nc.vector.tensor_mul(
                        att[:, : w2 * FD], egf[:, sl2], rbb[:, : w2 * FD]
                    )
                    gt = gtp.tile([128, 2 * FD], BF, tag="gt")
                    attx_mul(gt[:, : w2 * FD], att[:, : w2 * FD], xmf[:, sl2])
                    for i in range(w2):
                        kk = k0 + j + i
                        nc.tensor.matmul(
                            acc,
                            sf_sb,
                            gt[:, i * FD : (i + 1) * FD],
                            start=(kk == 0),
                            stop=(kk == NBLK - 1),
                        )
            elif fin_batch:
                # Defer the fin matmuls so all of a group's run back-to-back
                # with a single stationary (one LDWEIGHTS for sf).
                gts = []
                for j in range(gs):
                    rbb = rbpool.tile([128, FD], F32)
                    nc.tensor.matmul(
                        rbb,
                        sc_sb[: 4 * gs, j * 128 : (j + 1) * 128],
                        rb[: 4 * gs],
                        start=True,
                        stop=True,
                    )
                    att = attp.tile([128, FD], BF)
                    nc.vector.tensor_mul(att, eg[:, j], rbb)
                    gt = gtp.tile([128, FD], BF)
                    attx_mul(gt, att, xm[:, j])
                    gts.append(gt)
                for j, gt in enumerate(gts):
                    kk = k0 + j
                    nc.tensor.matmul(
                        acc,
                        sf_sb,
                        gt,
                        start=(kk == 0),
                        stop=(kk == NBLK - 1),
                    )
            else:
                for j in range(gs):
                    kk = k0 + j
                    rbb = rbpool.tile([128, FD], F32)
                    nc.tensor.matmul(
                        rbb,
                        sc_sb[: 4 * gs, j * 128 : (j + 1) * 128],
                        rb[: 4 * gs],
                        start=True,
                        stop=True,
                    )
                    att = attp.tile([128, FD], BF)
                    nc.vector.tensor_mul(att, eg[:, j], rbb)
                    gt = gtp.tile([128, FD], BF)
                    attx_mul(gt, att, xm[:, j])
                    nc.tensor.matmul(
                        acc,
                        sf_sb,
                        gt,
                        start=(kk == 0),
                        stop=(kk == NBLK - 1),
                    )

        # Software pipeline: phase2 of the previous group is emitted between
        # the DMA and the compute of the current group.  `repeat` re-runs the
        # whole body (for steady-state wall-clock benchmarking).
        prev = None
        dmas = first_dmas
        for rep in range(repeat):
            k0 = 0
            for gi, gs in enumerate(sched):
                nxt = None
                if gi + 1 < len(sched):
                    nxt = dma_in(k0 + gs, sched[gi + 1])
                elif rep + 1 < repeat:
                    nxt = dma_in(0, sched[0])
                if prev is not None:
                    phase2(
                        *prev, attx_pool=(gi - 1 >= len(sched) - attx_pool_tail)
                    )
                cur = phase1(k0, gs, *dmas, force_dve_mask=(gi < mask_dve_groups))
                prev = (k0, gs, *cur)
                k0 += gs
                dmas = nxt
        phase2(*prev, attx_pool=(attx_pool_tail > 0))

        out_sb = singles.tile([32, FD], F32)
        nc.vector.tensor_copy(out_sb, acc)
        nc.sync.dma_start(out=o_d[:, :], in_=out_sb)

    nc.compile()
    return nc


def _prep_inputs(inputs, mask, W, b):
    """Host-side shard + layout permutation (pure data movement + bf16 cast)."""
    # (B,T,S) -> (S,T,B), then split S into (core, blk, js).
    xt = np.ascontiguousarray(inputs.transpose(2, 1, 0)).astype(BF16)
    mt = np.ascontiguousarray(mask.transpose(2, 1, 0)).astype(BF16)

    def to_tiles(a):  # (S,T,B) -> (cores, NBLK, 128, B)
        return np.ascontiguousarray(a.reshape(NCORES, NBLK, 4 * T, B))

    x_hw = to_tiles(xt)
    m_hw = to_tiles(mt)

    # Block-diagonal weights: w_hw[c][js*32+t, kk*128 + js*32+u] = W[s,t,u]
    w4 = W.astype(BF16).reshape(NCORES, NBLK, 4, T, T)  # [c, kk, js, t, u]
    w_hw = np.zeros((NCORES, 4, T, NBLK, 4, T), dtype=BF16)
    for js in range(4):
        w_hw[:, js, :, :, js, :] = w4[:, :, js].transpose(0, 2, 1, 3)
    w_hw = np.ascontiguousarray(w_hw.reshape(NCORES, 128, NBLK * 128))

    # Bias: bb_hw[c][js*32+u, kk] = b[s,u]
    b4 = b.astype(np.float32).reshape(NCORES, NBLK, 4, T)  # [c, kk, js, u]
    bb_hw = np.ascontiguousarray(
        b4.transpose(0, 2, 3, 1).reshape(NCORES, 128, NBLK)
    )

    # Selector constants (shared by all cores).
    sd = np.zeros((4, T, GSZ, 4 * GSZ), dtype=BF16)
    for js in range(4):
        for k in range(GSZ):
            sd[js, :, k, 4 * k + js] = 1
    sd = np.ascontiguousarray(sd.reshape(128, GSZ * 4 * GSZ))

    sc = np.zeros((4 * GSZ, GSZ, 4, T), dtype=BF16)
    for k in range(GSZ):
        for js in range(4):
            sc[4 * k + js, k, js, :] = 1
    sc = np.ascontiguousarray(sc.reshape(4 * GSZ, GSZ * 128))

    sf = np.ascontiguousarray(np.tile(np.eye(T, dtype=BF16), (4, 1)))

    in_maps = []
    for c in range(NCORES):
        in_maps.append(
            {
                "x": x_hw[c],
                "m": m_hw[c],
                "w": w_hw[c],
                "bb": bb_hw[c],
                "sd": sd,
                "sc": sc,
                "sf": sf,
            }
        )
    return in_maps


def run(inputs, mask, W, b, trace=False, want_res=False, **build_kwargs):
    """Run on the 8 NeuronCores; returns (out, exec_time_ns[, results])."""
    from concourse.bass_utils import run_bass_kernel_spmd

    key = ("nc", tuple(sorted(build_kwargs.items())))
    if key not in _cache:
        _cache[key] = _build_nc(**build_kwargs)
    nc = _cache[key]
    in_maps = _prep_inputs(inputs, mask, W, b)
    res = run_bass_kernel_spmd(
        nc, in_maps, core_ids=list(range(NCORES)), trace=trace
    )
    partial = np.zeros((T, FD), dtype=np.float64)
    for r in res.results:
        partial += r["out"].astype(np.float64)
    out = np.ascontiguousarray(partial.T.astype(np.float32))  # (B, T)
    if want_res:
        return out, res.exec_time_ns, res
    return out, res.exec_time_ns


def kernel(inputs, mask, W, b):
    out, _ = run(inputs, mask, W, b, trace=False)
    return out


# revision 41
# speedup vs baseline: 67.2228x; 67.2228x over previous
"""Trainium2 Bass kernel for the 3D attention layer.

Math (reference):
    x[b,s,t] = inputs[b,t,s] * mask[b,t,s]
    z[b,s,u] = sum_t x[b,s,t] * W[s,t,u] + bias[s,u]
    e = exp(tanh(z))
    att[b,s,u] = e / (sum_u' e[b,s,u'] + eps)
    out[b,u] = sum_s att[b,s,u] * x[b,s,u]

Distribution: S (=2048) is sharded across 8 NeuronCores (256 positions per
core).  Each core computes a partial out[b,u] over its S-shard; the host sums
the 8 partials (the only cross-core reduction, 64 KB per core).

Per-core layout ("t-layout"): tiles are [partition=(js,t) or (js,u)][free=b]
where 4 consecutive s-positions (js=0..3) are packed into the 128 partitions
(4*32).  The per-position (T,T) matmuls become block-diagonal 128x128
matmuls with b=512 as the moving free dim.  The softmax denominator
(a partition-direction reduction) and its broadcast back across partitions
are done on the TensorEngine with constant 0/1 selector matrices; the
reciprocal itself runs on the small stacked [4*gs,512] tensor only.

All compute-facing tensors are bf16 (TensorEngine full rate, DVE 2x mode);
PSUM accumulation is fp32.  Work is emitted in variable-size groups of
position-blocks (small groups first/last to fill/drain the pipeline).
"""

import sys

for _p in ("/opt/trn_rl_repo", "/opt/pypackages"):
    if _p not in sys.path:
        sys.path.insert(0, _p)

import numpy as np
import ml_dtypes

BF16 = ml_dtypes.bfloat16

# Problem shape (hardcoded per spec).
B, T, S = 512, 32, 2048
NCORES = 8
S_LOC = S // NCORES       # 256 positions per core
NBLK = S_LOC // 4         # 64 blocks of 4 positions
GSZ = 8                   # max blocks per pipeline group (selector capacity)
FD = B                    # moving free dim (batch)

# Group schedule: sums to NBLK; small groups at both ends for pipeline
# fill/drain, GSZ-sized groups in steady state.
SCHED = [2, 2, 4, 8, 8, 8, 8, 8, 8, 4, 2, 2]
assert sum(SCHED) == NBLK

_cache = {}


def _build_nc(
    sched=tuple(SCHED),
    bufs_x=4,
    bufs_e=4,
    bufs_z=2,
    bufs_att=3,
    exp_chunk=2048,
    mask_engine="gpsimd",
    rb_copy_engine="vector",
    mask_dve_frac=0.0,
    group_mask=False,
    attx_pool_tail=0,
    w_chunks=4,
    repeat=1,
    mask_pair=False,
    att_pair=False,
    rb_bufs=2,
    mask_dve_groups=5,
    fin_batch=False,
    p2lag=2,
    den_bufs=3,
):
    """Build the single-core Bass graph (SPMD: same graph on all 8 cores)."""
    from contextlib import ExitStack

    import concourse.mybir as mybir
    from concourse import bacc
    from concourse.tile import TileContext

    sched = list(sched)
    assert sum(sched) == NBLK and max(sched) <= GSZ

    BF = mybir.dt.bfloat16
    F32 = mybir.dt.float32
    AF = mybir.ActivationFunctionType

    nc = bacc.Bacc(None, target_bir_lowering=False)
    x_d = nc.declare_dram_parameter("x", [NBLK, 128, FD], BF, isOutput=False)
    m_d = nc.declare_dram_parameter("m", [NBLK, 128, FD], BF, isOutput=False)
    w_d = nc.declare_dram_parameter("w", [128, NBLK * 128], BF, isOutput=False)
    bb_d = nc.declare_dram_parameter("bb", [128, NBLK], F32, isOutput=False)
    sd_d = nc.declare_dram_parameter("sd", [128, GSZ * 4 * GSZ], BF, isOutput=False)
    sc_d = nc.declare_dram_parameter("sc", [4 * GSZ, GSZ * 128], BF, isOutput=False)
    sf_d = nc.declare_dram_parameter("sf", [128, 32], BF, isOutput=False)
    o_d = nc.declare_dram_parameter("out", [32, FD], F32, isOutput=True)

    with TileContext(nc) as tc, ExitStack() as ctx:
        singles = ctx.enter_context(tc.tile_pool(name="singles", bufs=1))
        xp = ctx.enter_context(tc.tile_pool(name="xp", bufs=bufs_x))
        mp = ctx.enter_context(tc.tile_pool(name="mp", bufs=bufs_x))
        xmp = ctx.enter_context(tc.tile_pool(name="xmp", bufs=bufs_x))
        t1p = ctx.enter_context(tc.tile_pool(name="t1p", bufs=1))
        ep = ctx.enter_context(tc.tile_pool(name="ep", bufs=bufs_e))
        rp = ctx.enter_context(tc.tile_pool(name="rp", bufs=2))
        attp = ctx.enter_context(tc.tile_pool(name="attp", bufs=bufs_att))
        gtp = ctx.enter_context(tc.tile_pool(name="gtp", bufs=bufs_att))
        zpool = ctx.enter_context(tc.tile_pool(name="zp", bufs=bufs_z, space="PSUM"))
        dpool = ctx.enter_context(
            tc.tile_pool(name="dp", bufs=den_bufs, space="PSUM")
        )
        rbpool = ctx.enter_context(
            tc.tile_pool(name="rbp", bufs=rb_bufs, space="PSUM")
        )
        apool = ctx.enter_context(tc.tile_pool(name="ap", bufs=1, space="PSUM"))

        acc = apool.tile([32, FD], F32)
        mask_mul = (
            nc.gpsimd.tensor_mul if mask_engine == "gpsimd" else nc.vector.tensor_mul
        )

        def dma_in(k0, gs):
            xg = xp.tile([128, gs, FD], BF, tag="xg")
            nc.sync.dma_start(
                out=xg, in_=x_d[k0 : k0 + gs].rearrange("k p b -> p k b")
            )
            mg = mp.tile([128, gs, FD], BF, tag="mg")
            nc.sync.dma_start(
                out=mg, in_=m_d[k0 : k0 + gs].rearrange("k p b -> p k b")
            )
            return xg, mg

        # First-group input DMAs are issued before the (large) constant DMAs
        # so compute can start as soon as possible.
        first_dmas = dma_in(0, sched[0])

        # Constants, resident for the whole kernel.  W is split into chunks
        # loaded just-in-time inside the group loop so early-group input DMAs
        # aren't queued behind the full 2 MB weight transfer.
        w_sb = singles.tile([128, NBLK * 128], BF)
        wc = NBLK // w_chunks
        w_loaded = [False] * w_chunks

        def load_w_until(kmax):
            ci = 0
            while ci * wc < kmax:
                if not w_loaded[ci]:
                    nc.sync.dma_start(
                        out=w_sb[:, ci * wc * 128 : (ci + 1) * wc * 128],
                        in_=w_d[:, ci * wc * 128 : (ci + 1) * wc * 128],
                    )
                    w_loaded[ci] = True
                ci += 1

        load_w_until(sched[0])
        bb_sb = singles.tile([128, NBLK], F32)
        nc.sync.dma_start(out=bb_sb, in_=bb_d[:, :])
        sd_sb = singles.tile([128, GSZ * 4 * GSZ], BF)
        nc.sync.dma_start(out=sd_sb, in_=sd_d[:, :])
        sc_sb = singles.tile([4 * GSZ, GSZ * 128], BF)
        nc.sync.dma_start(out=sc_sb, in_=sc_d[:, :])
        sf_sb = singles.tile([128, 32], BF)
        nc.sync.dma_start(out=sf_sb, in_=sf_d[:, :])

        def phase1(k0, gs, xg, mg, force_dve_mask=False):
            g_mask_mul = nc.vector.tensor_mul if force_dve_mask else mask_mul
            xm = xmp.tile([128, gs, FD], BF, tag="xm")
            t1 = t1p.tile([128, GSZ * FD], BF)
            eg = ep.tile([128, gs, FD], BF, tag="eg")
            den = dpool.tile([4 * gs, FD], F32, tag="den")

            if group_mask:
                # One TT over the whole group amortizes per-op overhead.
                mask_mul(
                    xm.rearrange("p k b -> p (k b)"),
                    xg.rearrange("p k b -> p (k b)"),
                    mg.rearrange("p k b -> p (k b)"),
                )
            elif mask_pair:
                xmf = xm.rearrange("p k b -> p (k b)")
                xgf = xg.rearrange("p k b -> p (k b)")
                mgf = mg.rearrange("p k b -> p (k b)")
                for j in range(0, gs, 2):
                    w2 = min(2, gs - j)
                    sl2 = slice(j * FD, (j + w2) * FD)
                    mask_mul(xmf[:, sl2], xgf[:, sl2], mgf[:, sl2])
            n_dve_mask = int(round(gs * mask_dve_frac))
            for j in range(gs):
                kk = k0 + j
                if not group_mask and not mask_pair:
                    mm = (
                        nc.vector.tensor_mul
                        if j < n_dve_mask
                        else g_mask_mul
                    )
                    mm(xm[:, j], xg[:, j], mg[:, j])
                z = zpool.tile([128, FD], F32)
                nc.tensor.matmul(
                    z,
                    w_sb[:, kk * 128 : (kk + 1) * 128],
                    xm[:, j],
                    start=True,
                    stop=True,
                )
                # t1 = tanh(z + bias)  (bias folded into the ACT affine stage)
                nc.scalar.activation(
                    t1[:, j * FD : (j + 1) * FD],
                    z,
                    AF.Tanh,
                    bias=bb_sb[:, kk : kk + 1],
                )

            done = 0
            while done < gs * FD:
                csz = min(exp_chunk, gs * FD - done)
                nc.scalar.activation(
                    eg.rearrange("p k b -> p (k b)")[:, done : done + csz],
                    t1[:, done : done + csz],
                    AF.Exp,
                )
                done += csz

            for j in range(gs):
                nc.tensor.matmul(
                    den,
                    sd_sb[:, j * 4 * GSZ : j * 4 * GSZ + 4 * gs],
                    eg[:, j],
                    start=(j == 0),
                    stop=(j == gs - 1),
                )
            return xm, eg, den

        def phase2(k0, gs, xm, eg, den, attx_pool=False):
            rf = rp.tile([4 * GSZ, FD], F32, tag="rf")
            nc.vector.reciprocal_approx_fast(rf[: 4 * gs], den)
            rb = rp.tile([4 * GSZ, FD], BF, tag="rb")
            if rb_copy_engine == "scalar":
                nc.scalar.copy(rb[: 4 * gs], rf[: 4 * gs])
            else:
                nc.vector.tensor_copy(rb[: 4 * gs], rf[: 4 * gs])

            attx_mul = nc.gpsimd.tensor_mul if attx_pool else nc.vector.tensor_mul
            if att_pair:
                egf = eg.rearrange("p k b -> p (k b)")
                xmf = xm.rearrange("p k b -> p (k b)")
                for j in range(0, gs, 2):
                    w2 = min(2, gs - j)
                    rbb = rbpool.tile([128, 2 * FD], F32, tag="rbb")
                    for i in range(w2):
                        nc.tensor.matmul(
                            rbb[:, i * FD : (i + 1) * FD],
                            sc_sb[: 4 * gs, (j + i) * 128 : (j + i + 1) * 128],
                            rb[: 4 * gs],
                            start=True,
                            stop=True,
                        )
                    sl2 = slice(j * FD, (j + w2) * FD)
                    att = attp.tile([128, 2 * FD], BF, tag="att")
                    nc.vector.tensor_mul(
                        att[:, : w2 * FD], egf[:, sl2], rbb[:, : w2 * FD]
                    )
                    gt = gtp.tile([128, 2 * FD], BF, tag="gt")
                    attx_mul(gt[:, : w2 * FD], att[:, : w2 * FD], xmf[:, sl2])
                    for i in range(w2):
                        kk = k0 + j + i
                        nc.tensor.matmul(
                            acc,
                            sf_sb,
                            gt[:, i * FD : (i + 1) * FD],
                            start=(kk == 0),
                            stop=(kk == NBLK - 1),
                        )
            elif fin_batch:
                # Defer the fin matmuls so all of a group's run back-to-back
                # with a single stationary (one LDWEIGHTS for sf).
                gts = []
                for j in range(gs):
                    rbb = rbpool.tile([128, FD], F32)
                    nc.tensor.matmul(
                        rbb,
                        sc_sb[: 4 * gs, j * 128 : (j + 1) * 128],
                        rb[: 4 * gs],
                        start=True,
                        stop=True,
                    )
                    att = attp.tile([128, FD], BF)
                    nc.vector.tensor_mul(att, eg[:, j], rbb)
                    gt = gtp.tile([128, FD], BF)
                    attx_mul(gt, att, xm[:, j])
                    gts.append(gt)
                for j, gt in enumerate(gts):
                    kk = k0 + j
                    nc.tensor.matmul(
                        acc,
                        sf_sb,
                        gt,
                        start=(kk == 0),
                        stop=(kk == NBLK - 1),
                    )
            else:
                for j in range(gs):
                    kk = k0 + j
                    rbb = rbpool.tile([128, FD], F32)
                    nc.tensor.matmul(
                        rbb,
                        sc_sb[: 4 * gs, j * 128 : (j + 1) * 128],
                        rb[: 4 * gs],
                        start=True,
                        stop=True,
                    )
                    att = attp.tile([128, FD], BF)
                    nc.vector.tensor_mul(att, eg[:, j], rbb)
                    gt = gtp.tile([128, FD], BF)
                    attx_mul(gt, att, xm[:, j])
                    nc.tensor.matmul(
                        acc,
                        sf_sb,
                        gt,
                        start=(kk == 0),
                        stop=(kk == NBLK - 1),
                    )

        # Software pipeline.  p2lag=0: phase2 of the previous group is
        # emitted between the DMA and the compute of the current group.
        # p2lag>=1: phase2 of group g-p2lag is emitted AFTER phase1 of group
        # g, so a late phase2 input (the reciprocal) can never head-of-line
        # block the next group's matmuls in the in-order PE stream.
        # `repeat` re-runs the whole body (for wall-clock benchmarking).
        pending = []
        dmas = first_dmas
        for rep in range(repeat):
            k0 = 0
            for gi, gs in enumerate(sched):
                nxt = None
                if gi + 1 < len(sched):
                    nxt = dma_in(k0 + gs, sched[gi + 1])
                    load_w_until(k0 + gs + sched[gi + 1])
                elif rep + 1 < repeat:
                    nxt = dma_in(0, sched[0])
                late = gi >= len(sched) - attx_pool_tail
                if p2lag == 0:
                    if pending:
                        phase2(*pending.pop(0), attx_pool=late)
                    cur = phase1(
                        k0, gs, *dmas, force_dve_mask=(gi < mask_dve_groups)
                    )
                    pending.append((k0, gs, *cur))
                else:
                    cur = phase1(
                        k0, gs, *dmas, force_dve_mask=(gi < mask_dve_groups)
                    )
                    pending.append((k0, gs, *cur))
                    if len(pending) > p2lag:
                        phase2(*pending.pop(0), attx_pool=late)
                k0 += gs
                dmas = nxt
        while pending:
            phase2(*pending.pop(0), attx_pool=(attx_pool_tail > 0))

        out_sb = singles.tile([32, FD], F32)
        nc.vector.tensor_copy(out_sb, acc)
        nc.sync.dma_start(out=o_d[:, :], in_=out_sb)

    nc.compile()
    return nc


def _prep_inputs(inputs, mask, W, b):
    """Host-side shard + layout permutation (pure data movement + bf16 cast)."""
    # (B,T,S) -> (S,T,B), then split S into (core, blk, js).
    xt = np.ascontiguousarray(inputs.transpose(2, 1, 0)).astype(BF16)
    mt = np.ascontiguousarray(mask.transpose(2, 1, 0)).astype(BF16)

    def to_tiles(a):  # (S,T,B) -> (cores, NBLK, 128, B)
        return np.ascontiguousarray(a.reshape(NCORES, NBLK, 4 * T, B))

    x_hw = to_tiles(xt)
    m_hw = to_tiles(mt)

    # Block-diagonal weights: w_hw[c][js*32+t, kk*128 + js*32+u] = W[s,t,u]
    w4 = W.astype(BF16).reshape(NCORES, NBLK, 4, T, T)  # [c, kk, js, t, u]
    w_hw = np.zeros((NCORES, 4, T, NBLK, 4, T), dtype=BF16)
    for js in range(4):
        w_hw[:, js, :, :, js, :] = w4[:, :, js].transpose(0, 2, 1, 3)
    w_hw = np.ascontiguousarray(w_hw.reshape(NCORES, 128, NBLK * 128))

    # Bias: bb_hw[c][js*32+u, kk] = b[s,u]
    b4 = b.astype(np.float32).reshape(NCORES, NBLK, 4, T)  # [c, kk, js, u]
    bb_hw = np.ascontiguousarray(
        b4.transpose(0, 2, 3, 1).reshape(NCORES, 128, NBLK)
    )

    # Selector constants (shared by all cores).
    sd = np.zeros((4, T, GSZ, 4 * GSZ), dtype=BF16)
    for js in range(4):
        for k in range(GSZ):
            sd[js, :, k, 4 * k + js] = 1
    sd = np.ascontiguousarray(sd.reshape(128, GSZ * 4 * GSZ))

    sc = np.zeros((4 * GSZ, GSZ, 4, T), dtype=BF16)
    for k in range(GSZ):
        for js in range(4):
            sc[4 * k + js, k, js, :] = 1
    sc = np.ascontiguousarray(sc.reshape(4 * GSZ, GSZ * 128))

    sf = np.ascontiguousarray(np.tile(np.eye(T, dtype=BF16), (4, 1)))

    in_maps = []
    for c in range(NCORES):
        in_maps.append(
            {
                "x": x_hw[c],
                "m": m_hw[c],
                "w": w_hw[c],
                "bb": bb_hw[c],
                "sd": sd,
                "sc": sc,
                "sf": sf,
            }
        )
    return in_maps


def run(inputs, mask, W, b, trace=False, want_res=False, **build_kwargs):
    """Run on the 8 NeuronCores; returns (out, exec_time_ns[, results])."""
    from concourse.bass_utils import run_bass_kernel_spmd

    key = ("nc", tuple(sorted(build_kwargs.items())))
    if key not in _cache:
        _cache[key] = _build_nc(**build_kwargs)
    nc = _cache[key]
    in_maps = _prep_inputs(inputs, mask, W, b)
    res = run_bass_kernel_spmd(
        nc, in_maps, core_ids=list(range(NCORES)), trace=trace
    )
    partial = np.zeros((T, FD), dtype=np.float64)
    for r in res.results:
        partial += r["out"].astype(np.float64)
    out = np.ascontiguousarray(partial.T.astype(np.float32))  # (B, T)
    if want_res:
        return out, res.exec_time_ns, res
    return out, res.exec_time_ns


def kernel(inputs, mask, W, b):
    out, _ = run(inputs, mask, W, b, trace=False)
    return out
